# revision 1
# baseline (speedup 1.0000x reference)
"""Trainium2 Bass kernel for IntrinsicSignalSynthesizer.

Data-parallel over 8 NeuronCores: batch 16384 -> 8 x 2048 rows.
Feature-major dataflow: activations live as [feature_chunk(128), rows] tiles so
every matmul contracts over the partition dim with zero on-device transposes;
per-row reductions (sum_f) become ones-vector matmuls on the PE; the only
partition-dim reduction (max over 100 memory patterns) is done row-major with a
single PE transpose per row-tile to reassemble.
"""
import sys
sys.path.insert(0, '/opt/trn_rl_repo')

import numpy as np
import ml_dtypes

import concourse.bass as bass
import concourse.mybir as mybir
import concourse.tile as tile
from concourse.bass_utils import run_bass_kernel_spmd

BF16 = mybir.dt.bfloat16
F32 = mybir.dt.float32
AF = mybir.ActivationFunctionType
ALU = mybir.AluOpType

B, D = 16384, 1024
MEM = 100
NCORES = 8
ROWS = B // NCORES            # 2048 rows per core
NT = 512                      # rows per row-tile
NTILES = ROWS // NT           # 4
KD = D // 128                 # 8 feature chunks of prediction/actual

MAX_WAITS = 1


def _split_excess_waits(nc):
    # walrus CTRL encoding caps sync waits per instruction; the TileContext
    # tail drain can exceed that. Move excess waits onto preceding NoOps.
    for fn in nc.m.functions:
        for bb in fn.blocks:
            if not isinstance(bb, mybir.BasicBlock):
                continue
            insts = bb.instructions
            i = 0
            while i < len(insts):
                ins = insts[i]
                si = getattr(ins, 'sync_info', None)
                waits = list(si.on_wait) if si is not None and si.on_wait else []
                if len(waits) > MAX_WAITS:
                    chunks = [waits[j:j + MAX_WAITS]
                              for j in range(0, len(waits), MAX_WAITS)]
                    si.on_wait = chunks[-1]
                    new_ops = [
                        mybir.InstNoOp(
                            name=f"{ins.name}-waitsplit-{k}",
                            engine=ins.engine,
                            sync_info=mybir.SyncInfo(on_wait=ch, on_update=[]),
                            bass_nofuse=True,
                        )
                        for k, ch in enumerate(chunks[:-1])
                    ]
                    insts[i:i] = new_ops
                    i += len(new_ops)
                i += 1


def _mlp_layer(nc, pools, w_sb, bias_sb, x_tiles, kchunks, ofchunks, out_sb):
    """h = relu(W @ x + b) in feature-major layout.

    w_sb: [128, kchunks, 128*ofchunks] bf16; x_tiles(k) -> [128, NT] rhs AP;
    out_sb: [128, ofchunks, NT] bf16.
    """
    for j in range(ofchunks):
        ps = pools['mm'].tile([128, NT], F32, tag="mm")
        for k in range(kchunks):
            nc.tensor.matmul(ps, w_sb[:, k, j * 128:(j + 1) * 128], x_tiles(k),
                             start=(k == 0), stop=(k == kchunks - 1))
        nc.scalar.activation(out_sb[:, j, :], ps, AF.Relu,
                             bias=bias_sb[:, j:j + 1])


def _head(nc, pools, w_sb, h_sb, ofchunks):
    """z[1, NT] = w . h summed over features (PE ones-style reduction)."""
    ps = pools['vec'].tile([1, NT], F32, tag="vec")
    for j in range(ofchunks):
        nc.tensor.matmul(ps, w_sb[:, j, :], h_sb[:, j, :],
                         start=(j == 0), stop=(j == ofchunks - 1))
    return ps


def _softplus(nc, pools, z_ps, bias_sb, out_tag):
    """softplus(z + b) = Ln(1 + Exp(z + b)); both funcs share one ACT table set."""
    e = pools['sm'].tile([1, NT], F32, tag=out_tag + "_e")
    nc.scalar.activation(e, z_ps, AF.Exp, bias=bias_sb[0:1, 0:1])
    sp = pools['sm'].tile([1, NT], F32, tag=out_tag)
    nc.scalar.activation(sp, e, AF.Ln, bias=1.0)
    return sp


def build_kernel(reps: int = 1):
    nc = bass.Bass()

    pt_d = nc.dram_tensor("pt", [D, ROWS], BF16, kind="ExternalInput")
    at_d = nc.dram_tensor("at", [D, ROWS], BF16, kind="ExternalInput")
    wd_d = nc.dram_tensor("wd", [2 * D, D], BF16, kind="ExternalInput")
    wu_d = nc.dram_tensor("wu", [D, D // 2], BF16, kind="ExternalInput")
    wn_d = nc.dram_tensor("wn", [D, D // 2], BF16, kind="ExternalInput")
    wc1_d = nc.dram_tensor("wc1", [D, D // 4], BF16, kind="ExternalInput")
    wc2_d = nc.dram_tensor("wc2", [D // 4, D], BF16, kind="ExternalInput")
    wd2_d = nc.dram_tensor("wd2", [D, 1], BF16, kind="ExternalInput")
    wu2_d = nc.dram_tensor("wu2", [D // 2, 1], BF16, kind="ExternalInput")
    wn2_d = nc.dram_tensor("wn2", [D // 2, 1], BF16, kind="ExternalInput")
    mh_d = nc.dram_tensor("mh", [D, MEM], BF16, kind="ExternalInput")
    ones_d = nc.dram_tensor("ones", [128, 1], BF16, kind="ExternalInput")
    ident_d = nc.dram_tensor("ident", [128, 128], F32, kind="ExternalInput")
    bd1_d = nc.dram_tensor("bd1", [128, KD], F32, kind="ExternalInput")
    bu1_d = nc.dram_tensor("bu1", [128, 4], F32, kind="ExternalInput")
    bn1_d = nc.dram_tensor("bn1", [128, 4], F32, kind="ExternalInput")
    bc1_d = nc.dram_tensor("bc1", [128, 2], F32, kind="ExternalInput")
    bc2_d = nc.dram_tensor("bc2", [128, KD], F32, kind="ExternalInput")
    bh_d = nc.dram_tensor("bh", [1, 3], F32, kind="ExternalInput")  # d2,u2,n2
    out_d = nc.dram_tensor("out", [4, ROWS], F32, kind="ExternalOutput")

    with tile.TileContext(nc) as tc:
        pools = {}
        import contextlib
        ctx = contextlib.ExitStack()
        with ctx:
            W = ctx.enter_context(tc.tile_pool(name="weights", bufs=1))
            pools['io'] = ctx.enter_context(tc.tile_pool(name="io", bufs=2))
            pools['big'] = ctx.enter_context(tc.tile_pool(name="big", bufs=1))
            pools['sm'] = ctx.enter_context(tc.tile_pool(name="sm", bufs=1))
            pools['mm'] = ctx.enter_context(
                tc.tile_pool(name="mmp", bufs=3, space="PSUM"))
            pools['vec'] = ctx.enter_context(
                tc.tile_pool(name="vecp", bufs=2, space="PSUM"))
            pools['simp'] = ctx.enter_context(
                tc.tile_pool(name="simp", bufs=1, space="PSUM"))
            pools['tr'] = ctx.enter_context(
                tc.tile_pool(name="trp", bufs=1, space="PSUM"))

            # resident weights (loaded once)
            wd = W.tile([128, 16, D], BF16)
            nc.sync.dma_start(wd, wd_d.rearrange("(k p) m -> p k m", p=128))
            wu = W.tile([128, KD, D // 2], BF16)
            nc.sync.dma_start(wu, wu_d.rearrange("(k p) m -> p k m", p=128))
            wn = W.tile([128, KD, D // 2], BF16)
            nc.sync.dma_start(wn, wn_d.rearrange("(k p) m -> p k m", p=128))
            wc1 = W.tile([128, KD, D // 4], BF16)
            nc.sync.dma_start(wc1, wc1_d.rearrange("(k p) m -> p k m", p=128))
            wc2 = W.tile([128, 2, D], BF16)
            nc.sync.dma_start(wc2, wc2_d.rearrange("(k p) m -> p k m", p=128))
            wd2 = W.tile([128, KD, 1], BF16)
            nc.sync.dma_start(wd2, wd2_d.rearrange("(k p) m -> p k m", p=128))
            wu2 = W.tile([128, 4, 1], BF16)
            nc.sync.dma_start(wu2, wu2_d.rearrange("(k p) m -> p k m", p=128))
            wn2 = W.tile([128, 4, 1], BF16)
            nc.sync.dma_start(wn2, wn2_d.rearrange("(k p) m -> p k m", p=128))
            mh = W.tile([128, KD, MEM], BF16)
            nc.sync.dma_start(mh, mh_d.rearrange("(k p) m -> p k m", p=128))
            ones = W.tile([128, 1], BF16)
            nc.sync.dma_start(ones, ones_d[:])
            ident = W.tile([128, 128], F32)
            nc.sync.dma_start(ident, ident_d[:])
            bd1 = W.tile([128, KD], F32)
            nc.sync.dma_start(bd1, bd1_d[:])
            bu1 = W.tile([128, 4], F32)
            nc.sync.dma_start(bu1, bu1_d[:])
            bn1 = W.tile([128, 4], F32)
            nc.sync.dma_start(bn1, bn1_d[:])
            bc1 = W.tile([128, 2], F32)
            nc.sync.dma_start(bc1, bc1_d[:])
            bc2 = W.tile([128, KD], F32)
            nc.sync.dma_start(bc2, bc2_d[:])
            bh = W.tile([1, 3], F32)
            nc.sync.dma_start(bh, bh_d[:])

            for _ in range(reps):
                for t in range(NTILES):
                    rs = slice(t * NT, (t + 1) * NT)

                    pt = pools['io'].tile([128, KD, NT], BF16, tag="pt")
                    nc.sync.dma_start(
                        pt, pt_d[:, rs].rearrange("(k p) r -> p k r", p=128))
                    at = pools['io'].tile([128, KD, NT], BF16, tag="at")
                    nc.sync.dma_start(
                        at, at_d[:, rs].rearrange("(k p) r -> p k r", p=128))

                    # --- dissonance: relu([p;a] @ Wd1) -> head -> softplus
                    hd = pools['big'].tile([128, KD, NT], BF16, tag="hd")
                    _mlp_layer(nc, pools, wd, bd1,
                               lambda k: pt[:, k, :] if k < KD else at[:, k - KD, :],
                               16, KD, hd)
                    zd = _head(nc, pools, wd2, hd, KD)
                    spD = _softplus(nc, pools, zd, bh[0:1, 0:1], "spD")

                    # --- uncertainty MLP
                    hu = pools['big'].tile([128, 4, NT], BF16, tag="hu")
                    _mlp_layer(nc, pools, wu, bu1, lambda k: pt[:, k, :],
                               KD, 4, hu)
                    zu = _head(nc, pools, wu2, hu, 4)
                    spU = _softplus(nc, pools, zu, bh[0:1, 1:2], "spU")

                    # --- entropy: H = ln Z - (sum e*x)/Z  (no max needed)
                    e = pools['big'].tile([128, KD, NT], BF16, tag="e")
                    nc.scalar.activation(e, pt, AF.Exp)
                    ex = pools['big'].tile([128, KD, NT], BF16, tag="ex")
                    nc.vector.tensor_mul(ex, e, pt)
                    zZ = pools['vec'].tile([1, NT], F32, tag="vec")
                    for k in range(KD):
                        nc.tensor.matmul(zZ, ones, e[:, k, :],
                                         start=(k == 0), stop=(k == KD - 1))
                    zS = pools['vec'].tile([1, NT], F32, tag="vec")
                    for k in range(KD):
                        nc.tensor.matmul(zS, ones, ex[:, k, :],
                                         start=(k == 0), stop=(k == KD - 1))
                    lnZ = pools['sm'].tile([1, NT], F32, tag="lnZ")
                    nc.scalar.activation(lnZ, zZ, AF.Ln)
                    iZ = pools['sm'].tile([1, NT], F32, tag="iZ")
                    nc.vector.reciprocal(iZ, zZ)
                    sz = pools['sm'].tile([1, NT], F32, tag="sz")
                    nc.vector.tensor_mul(sz, zS, iZ)
                    hent = pools['sm'].tile([1, NT], F32, tag="hent")
                    nc.vector.tensor_tensor(hent, lnZ, sz, ALU.subtract)
                    unc = pools['sm'].tile([1, NT], F32, tag="unc")
                    nc.vector.scalar_tensor_tensor(
                        unc, hent, 0.1, spU, op0=ALU.mult, op1=ALU.add)

                    # --- novelty: memory part (row-major sims) + neural part
                    asq = pools['big'].tile([128, KD, NT], BF16, tag="asq")
                    nc.vector.tensor_mul(asq, at, at)
                    na2 = pools['vec'].tile([1, NT], F32, tag="vec")
                    for k in range(KD):
                        nc.tensor.matmul(na2, ones, asq[:, k, :],
                                         start=(k == 0), stop=(k == KD - 1))
                    lnA = pools['sm'].tile([1, NT], F32, tag="lnA")
                    nc.scalar.activation(lnA, na2, AF.Ln)
                    ia = pools['sm'].tile([1, NT], F32, tag="ia")
                    nc.scalar.activation(ia, lnA, AF.Exp, scale=-0.5)

                    raw4 = pools['sm'].tile([128, 4], F32, tag="raw4")
                    for s in range(4):
                        pss = pools['simp'].tile([128, MEM], F32, tag="simp")
                        for k in range(KD):
                            nc.tensor.matmul(
                                pss, at[:, k, s * 128:(s + 1) * 128],
                                mh[:, k, :],
                                start=(k == 0), stop=(k == KD - 1))
                        nc.vector.reduce_max(raw4[:, s:s + 1], pss,
                                             axis=mybir.AxisListType.X)
                    pst = pools['tr'].tile([4, 128], F32, tag="tr")
                    nc.tensor.transpose(pst, raw4, ident)
                    st = pools['sm'].tile([4, 128], F32, tag="st")
                    nc.scalar.copy(st, pst)
                    mem_raw = pools['sm'].tile([1, 4, 128], F32, tag="mem_raw")
                    nc.sync.dma_start(mem_raw, st)

                    hn = pools['big'].tile([128, 4, NT], BF16, tag="hn")
                    _mlp_layer(nc, pools, wn, bn1, lambda k: at[:, k, :],
                               KD, 4, hn)
                    zn = _head(nc, pools, wn2, hn, 4)
                    spN = _softplus(nc, pools, zn, bh[0:1, 2:3], "spN")

                    mr = mem_raw.rearrange("o s c -> o (s c)")
                    cos = pools['sm'].tile([1, NT], F32, tag="cos")
                    nc.vector.tensor_mul(cos, mr, ia)
                    # nov = 0.7*(1-cos) + 0.3*spN = (cos*-0.7 + 0.3*spN) + 0.7
                    spN3 = pools['sm'].tile([1, NT], F32, tag="spN3")
                    nc.vector.tensor_scalar_mul(spN3, spN, 0.3)
                    nov = pools['sm'].tile([1, NT], F32, tag="nov")
                    nc.vector.scalar_tensor_tensor(
                        nov, cos, -0.7, spN3, op0=ALU.mult, op1=ALU.add)
                    nc.vector.tensor_scalar_add(nov, nov, 0.7)

                    # --- compression
                    hc = pools['big'].tile([128, 2, NT], BF16, tag="hc")
                    _mlp_layer(nc, pools, wc1, bc1, lambda k: pt[:, k, :],
                               KD, 2, hc)
                    dsq = pools['big'].tile([128, KD, NT], BF16, tag="dsq")
                    for j in range(KD):
                        psr = pools['mm'].tile([128, NT], F32, tag="mm")
                        for k in range(2):
                            nc.tensor.matmul(psr, wc2[:, k, j * 128:(j + 1) * 128],
                                             hc[:, k, :],
                                             start=(k == 0), stop=(k == 1))
                        dj = pools['sm'].tile([128, NT], BF16, tag="dj")
                        # dj = (recon + bc2) - p   (sign-flipped diff; squared next)
                        nc.vector.scalar_tensor_tensor(
                            dj, psr, bc2[:, j:j + 1], pt[:, j, :],
                            op0=ALU.add, op1=ALU.subtract)
                        nc.vector.tensor_mul(dsq[:, j, :], dj, dj)
                    msum = pools['vec'].tile([1, NT], F32, tag="vec")
                    for k in range(KD):
                        nc.tensor.matmul(msum, ones, dsq[:, k, :],
                                         start=(k == 0), stop=(k == KD - 1))
                    comp = pools['sm'].tile([1, NT], F32, tag="comp")
                    nc.vector.tensor_scalar_mul(comp, msum, 1.0 / D)

                    nc.sync.dma_start(out_d[0:1, rs], spD)
                    nc.sync.dma_start(out_d[1:2, rs], unc)
                    nc.sync.dma_start(out_d[2:3, rs], nov)
                    nc.sync.dma_start(out_d[3:4, rs], comp)

    _split_excess_waits(nc)
    return nc


def _prep_inputs(prediction, actual, pattern_memory,
                 W_d1, b_d1, W_d2, b_d2, W_u1, b_u1, W_u2, b_u2,
                 W_n1, b_n1, W_n2, b_n2, W_c1, b_c1, W_c2, b_c2):
    bf = ml_dtypes.bfloat16

    def t_bf(a):  # transposed contiguous bf16
        return np.ascontiguousarray(np.asarray(a, np.float32).T).astype(bf)

    mnorm = np.maximum(np.linalg.norm(
        np.asarray(pattern_memory, np.float32), axis=1), 1e-8)
    mhat = np.asarray(pattern_memory, np.float32) / mnorm[:, None]

    def fold_bias(b, chunks):
        return np.ascontiguousarray(
            np.asarray(b, np.float32).reshape(chunks, 128).T)

    shared = {
        "wd": t_bf(W_d1), "wu": t_bf(W_u1), "wn": t_bf(W_n1),
        "wc1": t_bf(W_c1), "wc2": t_bf(W_c2),
        "wd2": t_bf(W_d2), "wu2": t_bf(W_u2), "wn2": t_bf(W_n2),
        "mh": t_bf(mhat),
        "ones": np.ones((128, 1), bf),
        "ident": np.eye(128, dtype=np.float32),
        "bd1": fold_bias(b_d1, KD), "bu1": fold_bias(b_u1, 4),
        "bn1": fold_bias(b_n1, 4), "bc1": fold_bias(b_c1, 2),
        "bc2": fold_bias(b_c2, KD),
        "bh": np.asarray([[float(b_d2[0]), float(b_u2[0]), float(b_n2[0])]],
                         np.float32),
    }
    p32 = np.asarray(prediction, np.float32)
    a32 = np.asarray(actual, np.float32)
    in_maps = []
    for c in range(NCORES):
        rows = slice(c * ROWS, (c + 1) * ROWS)
        m = dict(shared)
        m["pt"] = np.ascontiguousarray(p32[rows].T).astype(bf)
        m["at"] = np.ascontiguousarray(a32[rows].T).astype(bf)
        in_maps.append(m)
    return in_maps


_NC_CACHE = {}


def kernel(**inputs) -> np.ndarray:
    in_maps = _prep_inputs(**inputs)
    if 'nc' not in _NC_CACHE:
        _NC_CACHE['nc'] = build_kernel(reps=1)
    nc = _NC_CACHE['nc']
    res = run_bass_kernel_spmd(nc, in_maps, core_ids=list(range(NCORES)))
    out = np.empty((B, 4), np.float32)
    for c in range(NCORES):
        out[c * ROWS:(c + 1) * ROWS, :] = res.results[c]["out"].T
    return out



# revision 21
# speedup vs baseline: 1.0783x; 1.0783x over previous
"""Trainium2 Bass kernel for IntrinsicSignalSynthesizer.

Data-parallel over 8 NeuronCores: batch 16384 -> 8 x 2048 rows.

v2 design notes:
- MLP matmuls stay feature-major ([128 feat, kchunk, rows] tiles) so every
  matmul contracts over the partition dim at the bf16 stream roofline.
- Per-row reductions that the baseline did as PE ones-matmuls (sum of e^p,
  sum p*e^p, sum a^2) moved OFF the PE: inputs are DMA'd a second time in
  row-major layout and reduced along the free dim via ACT accum_out /
  DVE tensor_tensor_reduce.
- Per-row scalar finishing is done row-major [128 rows, 16 subblocks] per
  core (one instruction per op for all 2048 rows) instead of [1, 512] per
  row-tile; head/msum scalars are transposed into that layout on the PE
  (4 tiny transposes per row-tile).
- Weight DMAs are ordered so compute (sims, entropy, small MLPs) starts
  while the big dissonance weight streams in.
"""
import sys
sys.path.insert(0, '/opt/trn_rl_repo')

import numpy as np
import ml_dtypes

import concourse.bass as bass
import concourse.mybir as mybir
import concourse.tile as tile
from concourse.bass_utils import run_bass_kernel_spmd

BF16 = mybir.dt.bfloat16
F32 = mybir.dt.float32
AF = mybir.ActivationFunctionType
ALU = mybir.AluOpType
AX = mybir.AxisListType

B, D = 16384, 1024
MEM = 100
NCORES = 8
ROWS = B // NCORES            # 2048 rows per core
NT = 512                      # rows per row-tile
NTILES = ROWS // NT           # 4
NSUB = ROWS // 128            # 16 row-subblocks per core
KD = D // 128                 # 8 feature chunks

MAX_WAITS = 1


def _split_excess_waits(nc):
    # walrus CTRL encoding caps sync waits per instruction; the TileContext
    # tail drain can exceed that. Move excess waits onto preceding NoOps.
    for fn in nc.m.functions:
        for bb in fn.blocks:
            if not isinstance(bb, mybir.BasicBlock):
                continue
            insts = bb.instructions
            i = 0
            while i < len(insts):
                ins = insts[i]
                si = getattr(ins, 'sync_info', None)
                waits = list(si.on_wait) if si is not None and si.on_wait else []
                if len(waits) > MAX_WAITS:
                    chunks = [waits[j:j + MAX_WAITS]
                              for j in range(0, len(waits), MAX_WAITS)]
                    si.on_wait = chunks[-1]
                    new_ops = [
                        mybir.InstNoOp(
                            name=f"{ins.name}-waitsplit-{k}",
                            engine=ins.engine,
                            sync_info=mybir.SyncInfo(on_wait=ch, on_update=[]),
                            bass_nofuse=True,
                        )
                        for k, ch in enumerate(chunks[:-1])
                    ]
                    insts[i:i] = new_ops
                    i += len(new_ops)
                i += 1


def build_kernel(reps: int = 1):
    nc = bass.Bass()

    pt_d = nc.dram_tensor("pt", [D, ROWS], BF16, kind="ExternalInput")
    at_d = nc.dram_tensor("at", [D, ROWS], BF16, kind="ExternalInput")
    prm_d = nc.dram_tensor("prm", [128, NSUB, D], BF16, kind="ExternalInput")
    arm_d = nc.dram_tensor("arm", [128, NSUB, D], BF16, kind="ExternalInput")
    wd_d = nc.dram_tensor("wd", [2 * D, D], BF16, kind="ExternalInput")
    wu_d = nc.dram_tensor("wu", [D, D // 2], BF16, kind="ExternalInput")
    wn_d = nc.dram_tensor("wn", [D, D // 2], BF16, kind="ExternalInput")
    wc1_d = nc.dram_tensor("wc1", [D, D // 4], BF16, kind="ExternalInput")
    wc2_d = nc.dram_tensor("wc2", [D // 4, D], BF16, kind="ExternalInput")
    wd2_d = nc.dram_tensor("wd2", [D, 1], BF16, kind="ExternalInput")
    wu2_d = nc.dram_tensor("wu2", [D // 2, 1], BF16, kind="ExternalInput")
    wn2_d = nc.dram_tensor("wn2", [D // 2, 1], BF16, kind="ExternalInput")
    mh_d = nc.dram_tensor("mh", [D, MEM], BF16, kind="ExternalInput")
    ones_d = nc.dram_tensor("ones", [128, 1], BF16, kind="ExternalInput")
    ident_d = nc.dram_tensor("ident", [128, 128], F32, kind="ExternalInput")
    bd1_d = nc.dram_tensor("bd1", [128, KD], F32, kind="ExternalInput")
    bu1_d = nc.dram_tensor("bu1", [128, 4], F32, kind="ExternalInput")
    bn1_d = nc.dram_tensor("bn1", [128, 4], F32, kind="ExternalInput")
    bc1_d = nc.dram_tensor("bc1", [128, 2], F32, kind="ExternalInput")
    bc2_d = nc.dram_tensor("bc2", [128, KD], F32, kind="ExternalInput")
    bh_d = nc.dram_tensor("bh", [128, 3], F32, kind="ExternalInput")  # d2,u2,n2
    out_d = nc.dram_tensor("out", [4, ROWS], F32, kind="ExternalOutput")

    with tile.TileContext(nc) as tc:
        import contextlib
        ctx = contextlib.ExitStack()
        with ctx:
            W = ctx.enter_context(tc.tile_pool(name="weights", bufs=1))
            io = ctx.enter_context(tc.tile_pool(name="io", bufs=2))
            big = ctx.enter_context(tc.tile_pool(name="big", bufs=2))
            sm = ctx.enter_context(tc.tile_pool(name="sm", bufs=2))
            acc = ctx.enter_context(tc.tile_pool(name="acc", bufs=1))
            mm = ctx.enter_context(tc.tile_pool(name="mmp", bufs=3, space="PSUM"))
            vec = ctx.enter_context(tc.tile_pool(name="vecp", bufs=2, space="PSUM"))
            simp = ctx.enter_context(tc.tile_pool(name="simp", bufs=1, space="PSUM"))
            trp = ctx.enter_context(tc.tile_pool(name="trp", bufs=1, space="PSUM"))

            # --- resident weights; emission order = DMA start order, so the
            # tensors needed by the first PE work go first and the big
            # dissonance weight streams in under early compute.
            mh = W.tile([128, KD, MEM], BF16)
            nc.sync.dma_start(mh, mh_d.rearrange("(k p) m -> p k m", p=128))
            wu = W.tile([128, KD, D // 2], BF16)
            nc.sync.dma_start(wu, wu_d.rearrange("(k p) m -> p k m", p=128))
            wn = W.tile([128, KD, D // 2], BF16)
            nc.sync.dma_start(wn, wn_d.rearrange("(k p) m -> p k m", p=128))
            wc1 = W.tile([128, KD, D // 4], BF16)
            nc.sync.dma_start(wc1, wc1_d.rearrange("(k p) m -> p k m", p=128))
            wc2 = W.tile([128, 2, D], BF16)
            nc.sync.dma_start(wc2, wc2_d.rearrange("(k p) m -> p k m", p=128))
            wd2 = W.tile([128, KD, 1], BF16)
            nc.sync.dma_start(wd2, wd2_d.rearrange("(k p) m -> p k m", p=128))
            wu2 = W.tile([128, 4, 1], BF16)
            nc.sync.dma_start(wu2, wu2_d.rearrange("(k p) m -> p k m", p=128))
            wn2 = W.tile([128, 4, 1], BF16)
            nc.sync.dma_start(wn2, wn2_d.rearrange("(k p) m -> p k m", p=128))
            ones = W.tile([128, 1], BF16)
            nc.sync.dma_start(ones, ones_d[:])
            ident = W.tile([128, 128], F32)
            nc.sync.dma_start(ident, ident_d[:])
            bd1 = W.tile([128, KD], F32)
            nc.sync.dma_start(bd1, bd1_d[:])
            bu1 = W.tile([128, 4], F32)
            nc.sync.dma_start(bu1, bu1_d[:])
            bn1 = W.tile([128, 4], F32)
            nc.sync.dma_start(bn1, bn1_d[:])
            bc1 = W.tile([128, 2], F32)
            nc.sync.dma_start(bc1, bc1_d[:])
            bc2 = W.tile([128, KD], F32)
            nc.sync.dma_start(bc2, bc2_d[:])
            bh = W.tile([128, 3], F32)
            nc.sync.dma_start(bh, bh_d[:])
            # dissonance weight, split per k-chunk so each accumulation step
            # only waits for its own chunk
            wd = []
            for k in range(16):
                wk = W.tile([128, D], BF16, tag=f"wdc{k}")
                nc.sync.dma_start(wk, wd_d[k * 128:(k + 1) * 128, :])
                wd.append(wk)

            for _ in range(reps):
                # per-core row-major scalar accumulators [128 rows, 16 subs]
                zZ = acc.tile([128, NSUB], F32, tag="zZ")
                zS = acc.tile([128, NSUB], F32, tag="zS")
                na2 = acc.tile([128, NSUB], F32, tag="na2")
                raw = acc.tile([128, NSUB], F32, tag="raw")
                zAll = acc.tile([128, NTILES, 4, 4], F32, tag="zAll")

                for t in range(NTILES):
                    rs = slice(t * NT, (t + 1) * NT)

                    at = io.tile([128, KD, NT], BF16, tag="at")
                    nc.sync.dma_start(
                        at, at_d[:, rs].rearrange("(k p) r -> p k r", p=128))
                    pt = io.tile([128, KD, NT], BF16, tag="pt")
                    nc.sync.dma_start(
                        pt, pt_d[:, rs].rearrange("(k p) r -> p k r", p=128))
                    prm = io.tile([128, 4, D], BF16, tag="prm")
                    nc.sync.dma_start(prm, prm_d[:, 4 * t:4 * t + 4, :])
                    arm = io.tile([128, 4, D], BF16, tag="arm")
                    nc.sync.dma_start(arm, arm_d[:, 4 * t:4 * t + 4, :])

                    # --- sims (needs only mh + at): row-major max cos sim
                    for s in range(4):
                        g = 4 * t + s
                        pss = simp.tile([128, MEM], F32, tag="simp")
                        for k in range(KD):
                            nc.tensor.matmul(
                                pss, at[:, k, s * 128:(s + 1) * 128],
                                mh[:, k, :],
                                start=(k == 0), stop=(k == KD - 1))
                        nc.vector.reduce_max(raw[:, g:g + 1], pss, axis=AX.X)

                    # --- entropy sums + ||a||^2, row-major free-dim reduces
                    for s in range(4):
                        g = 4 * t + s
                        e = sm.tile([128, D], BF16, tag="e")
                        nc.scalar.activation(e, prm[:, s, :], AF.Exp)
                        nc.vector.tensor_reduce(
                            zZ[:, g:g + 1], e, axis=AX.X, op=ALU.add)
                        e2 = sm.tile([128, D], BF16, tag="e2")
                        nc.vector.tensor_mul(e2, e, prm[:, s, :])
                        nc.vector.tensor_reduce(
                            zS[:, g:g + 1], e2, axis=AX.X, op=ALU.add)
                        asq = sm.tile([128, D], BF16, tag="asq")
                        nc.vector.tensor_mul(asq, arm[:, s, :], arm[:, s, :])
                        nc.vector.tensor_reduce(
                            na2[:, g:g + 1], asq, axis=AX.X, op=ALU.add)

                    # per-row scalars staged at quadrant-aligned partitions
                    # {0,32,64,96} (engine writes must be 32-aligned)
                    z4 = sm.tile([128, NT], F32, tag="z4")

                    # --- uncertainty MLP
                    hu = big.tile([128, 4, NT], BF16, tag="hu")
                    for j in range(4):
                        ps = mm.tile([128, NT], F32, tag="mm")
                        for k in range(KD):
                            nc.tensor.matmul(ps, wu[:, k, j * 128:(j + 1) * 128],
                                             pt[:, k, :],
                                             start=(k == 0), stop=(k == KD - 1))
                        nc.scalar.activation(hu[:, j, :], ps, AF.Relu,
                                             bias=bu1[:, j:j + 1])
                    zu = vec.tile([1, NT], F32, tag="vec")
                    for j in range(4):
                        nc.tensor.matmul(zu, wu2[:, j, :], hu[:, j, :],
                                         start=(j == 0), stop=(j == 3))
                    nc.scalar.copy(z4[32:33, :], zu)

                    # --- novelty MLP
                    hn = big.tile([128, 4, NT], BF16, tag="hn")
                    for j in range(4):
                        ps = mm.tile([128, NT], F32, tag="mm")
                        for k in range(KD):
                            nc.tensor.matmul(ps, wn[:, k, j * 128:(j + 1) * 128],
                                             at[:, k, :],
                                             start=(k == 0), stop=(k == KD - 1))
                        nc.scalar.activation(hn[:, j, :], ps, AF.Relu,
                                             bias=bn1[:, j:j + 1])
                    zn = vec.tile([1, NT], F32, tag="vec")
                    for j in range(4):
                        nc.tensor.matmul(zn, wn2[:, j, :], hn[:, j, :],
                                         start=(j == 0), stop=(j == 3))
                    nc.scalar.copy(z4[64:65, :], zn)

                    # --- compression
                    hc = big.tile([128, 2, NT], BF16, tag="hc")
                    for j in range(2):
                        ps = mm.tile([128, NT], F32, tag="mm")
                        for k in range(KD):
                            nc.tensor.matmul(ps, wc1[:, k, j * 128:(j + 1) * 128],
                                             pt[:, k, :],
                                             start=(k == 0), stop=(k == KD - 1))
                        nc.scalar.activation(hc[:, j, :], ps, AF.Relu,
                                             bias=bc1[:, j:j + 1])
                    ms = vec.tile([1, NT], F32, tag="vec")
                    for j in range(KD):
                        psr = mm.tile([128, NT], F32, tag="mm")
                        for k in range(2):
                            nc.tensor.matmul(psr, wc2[:, k, j * 128:(j + 1) * 128],
                                             hc[:, k, :],
                                             start=(k == 0), stop=(k == 1))
                        dj = sm.tile([128, NT], BF16, tag="dj")
                        # dj = (recon + bc2) - p  (sign-flipped diff; squared)
                        nc.vector.scalar_tensor_tensor(
                            dj, psr, bc2[:, j:j + 1], pt[:, j, :],
                            op0=ALU.add, op1=ALU.subtract)
                        dsq = sm.tile([128, NT], BF16, tag="dsq")
                        nc.scalar.activation(dsq, dj, AF.Square)
                        nc.tensor.matmul(ms, ones, dsq,
                                         start=(j == 0), stop=(j == KD - 1))
                    nc.vector.tensor_copy(z4[96:97, :], ms)

                    # --- dissonance
                    hd = big.tile([128, KD, NT], BF16, tag="hd")
                    for j in range(KD):
                        ps = mm.tile([128, NT], F32, tag="mm")
                        for k in range(16):
                            x = pt[:, k, :] if k < KD else at[:, k - KD, :]
                            nc.tensor.matmul(ps, wd[k][:, j * 128:(j + 1) * 128],
                                             x, start=(k == 0), stop=(k == 15))
                        nc.scalar.activation(hd[:, j, :], ps, AF.Relu,
                                             bias=bd1[:, j:j + 1])
                    zd = vec.tile([1, NT], F32, tag="vec")
                    for j in range(KD):
                        nc.tensor.matmul(zd, wd2[:, j, :], hd[:, j, :],
                                         start=(j == 0), stop=(j == KD - 1))
                    nc.scalar.copy(z4[0:1, :], zd)

                    # --- transpose the 4 per-row scalars into row-major.
                    # Full [128,128] transposes (K=128 is the only shape the
                    # backend handles); quantities land on cols {0,32,64,96},
                    # picked out by a strided DVE copy per subblock.
                    for s in range(4):
                        ztr = trp.tile([128, 4, 32], F32, tag="ztr")
                        nc.tensor.transpose(
                            ztr, z4[:, s * 128:(s + 1) * 128], ident)
                        nc.vector.tensor_copy(zAll[:, t, s, :], ztr[:, :, 0])

                # --- per-core finishing, all [128 rows, 16 subs]
                zdR = zAll[:, :, :, 0]
                zuR = zAll[:, :, :, 1]
                znR = zAll[:, :, :, 2]
                msR = zAll[:, :, :, 3]

                fin = acc.tile([128, 12, NSUB], F32, tag="fin")
                spD, spU, spN = fin[:, 0, :], fin[:, 1, :], fin[:, 2, :]
                tmp1, tmp2, tmp3 = fin[:, 3, :], fin[:, 4, :], fin[:, 5, :]
                uncR, novR, compR = fin[:, 6, :], fin[:, 7, :], fin[:, 8, :]

                # softplus(z + b) = Ln(1 + Exp(z + b))
                nc.scalar.activation(tmp1, zdR, AF.Exp, bias=bh[:, 0:1])
                nc.scalar.activation(spD, tmp1, AF.Ln, bias=1.0)
                nc.scalar.activation(tmp1, zuR, AF.Exp, bias=bh[:, 1:2])
                nc.scalar.activation(spU, tmp1, AF.Ln, bias=1.0)
                nc.scalar.activation(tmp1, znR, AF.Exp, bias=bh[:, 2:3])
                nc.scalar.activation(spN, tmp1, AF.Ln, bias=1.0)

                # uncertainty = spU + 0.1 * (lnZ - zS/Z)
                nc.scalar.activation(tmp1, zZ, AF.Ln)          # lnZ
                nc.vector.reciprocal(tmp2, zZ)                 # 1/Z
                nc.vector.tensor_mul(tmp3, zS, tmp2)           # zS/Z
                nc.vector.tensor_tensor(tmp1, tmp1, tmp3, ALU.subtract)
                nc.vector.scalar_tensor_tensor(
                    uncR, tmp1, 0.1, spU, op0=ALU.mult, op1=ALU.add)

                # novelty = 0.7*(1 - raw/||a||) + 0.3*spN
                nc.scalar.activation(tmp1, na2, AF.Ln)
                nc.scalar.activation(tmp2, tmp1, AF.Exp, scale=-0.5)  # 1/||a||
                nc.vector.tensor_mul(tmp3, raw, tmp2)          # cos
                nc.vector.tensor_scalar_mul(tmp1, spN, 0.3)
                nc.vector.scalar_tensor_tensor(
                    novR, tmp3, -0.7, tmp1, op0=ALU.mult, op1=ALU.add)
                nc.vector.tensor_scalar_add(novR, novR, 0.7)

                nc.vector.tensor_scalar_mul(compR, msR, 1.0 / D)

                # --- assemble [4, ROWS] output: transpose each signal
                for q, src in enumerate((spD, uncR, novR, compR)):
                    oT = trp.tile([NSUB, 128], F32, tag="oT")
                    nc.tensor.transpose(oT, src, ident)
                    ob = sm.tile([NSUB, 128], F32, tag="ob")
                    nc.scalar.copy(ob, oT)
                    nc.sync.dma_start(
                        out_d[q:q + 1, :].rearrange("a (s r) -> (a s) r", s=NSUB),
                        ob)

    _split_excess_waits(nc)
    return nc


def _prep_inputs(prediction, actual, pattern_memory,
                 W_d1, b_d1, W_d2, b_d2, W_u1, b_u1, W_u2, b_u2,
                 W_n1, b_n1, W_n2, b_n2, W_c1, b_c1, W_c2, b_c2):
    bf = ml_dtypes.bfloat16

    def t_bf(a):  # transposed contiguous bf16
        return np.ascontiguousarray(np.asarray(a, np.float32).T).astype(bf)

    mnorm = np.maximum(np.linalg.norm(
        np.asarray(pattern_memory, np.float32), axis=1), 1e-8)
    mhat = np.asarray(pattern_memory, np.float32) / mnorm[:, None]

    def fold_bias(b, chunks):
        return np.ascontiguousarray(
            np.asarray(b, np.float32).reshape(chunks, 128).T)

    bh = np.empty((128, 3), np.float32)
    bh[:, 0] = float(np.asarray(b_d2).reshape(-1)[0])
    bh[:, 1] = float(np.asarray(b_u2).reshape(-1)[0])
    bh[:, 2] = float(np.asarray(b_n2).reshape(-1)[0])

    shared = {
        "wd": t_bf(W_d1), "wu": t_bf(W_u1), "wn": t_bf(W_n1),
        "wc1": t_bf(W_c1), "wc2": t_bf(W_c2),
        "wd2": t_bf(W_d2), "wu2": t_bf(W_u2), "wn2": t_bf(W_n2),
        "mh": t_bf(mhat),
        "ones": np.ones((128, 1), bf),
        "ident": np.eye(128, dtype=np.float32),
        "bd1": fold_bias(b_d1, KD), "bu1": fold_bias(b_u1, 4),
        "bn1": fold_bias(b_n1, 4), "bc1": fold_bias(b_c1, 2),
        "bc2": fold_bias(b_c2, KD),
        "bh": bh,
    }
    p32 = np.asarray(prediction, np.float32)
    a32 = np.asarray(actual, np.float32)
    in_maps = []
    for c in range(NCORES):
        rows = slice(c * ROWS, (c + 1) * ROWS)
        m = dict(shared)
        m["pt"] = np.ascontiguousarray(p32[rows].T).astype(bf)
        m["at"] = np.ascontiguousarray(a32[rows].T).astype(bf)
        m["prm"] = np.ascontiguousarray(
            p32[rows].reshape(NSUB, 128, D).transpose(1, 0, 2)).astype(bf)
        m["arm"] = np.ascontiguousarray(
            a32[rows].reshape(NSUB, 128, D).transpose(1, 0, 2)).astype(bf)
        in_maps.append(m)
    return in_maps


_NC_CACHE = {}


def kernel(**inputs) -> np.ndarray:
    in_maps = _prep_inputs(**inputs)
    if 'nc' not in _NC_CACHE:
        _NC_CACHE['nc'] = build_kernel(reps=1)
    nc = _NC_CACHE['nc']
    res = run_bass_kernel_spmd(nc, in_maps, core_ids=list(range(NCORES)))
    out = np.empty((B, 4), np.float32)
    for c in range(NCORES):
        out[c * ROWS:(c + 1) * ROWS, :] = res.results[c]["out"].T
    return out


# revision 25
# speedup vs baseline: 1.0999x; 1.0200x over previous
"""Trainium2 Bass kernel for IntrinsicSignalSynthesizer.

Data-parallel over 8 NeuronCores: batch 16384 -> 8 x 2048 rows.

v2 design notes:
- MLP matmuls stay feature-major ([128 feat, kchunk, rows] tiles) so every
  matmul contracts over the partition dim at the bf16 stream roofline.
- Per-row reductions that the baseline did as PE ones-matmuls (sum of e^p,
  sum p*e^p, sum a^2) moved OFF the PE: inputs are DMA'd a second time in
  row-major layout and reduced along the free dim via ACT accum_out /
  DVE tensor_tensor_reduce.
- Per-row scalar finishing is done row-major [128 rows, 16 subblocks] per
  core (one instruction per op for all 2048 rows) instead of [1, 512] per
  row-tile; head/msum scalars are transposed into that layout on the PE
  (4 tiny transposes per row-tile).
- Weight DMAs are ordered so compute (sims, entropy, small MLPs) starts
  while the big dissonance weight streams in.
"""
import sys
sys.path.insert(0, '/opt/trn_rl_repo')

import numpy as np
import ml_dtypes

import concourse.bass as bass
import concourse.mybir as mybir
import concourse.tile as tile
from concourse.bass_utils import run_bass_kernel_spmd

BF16 = mybir.dt.bfloat16
F32 = mybir.dt.float32
AF = mybir.ActivationFunctionType
ALU = mybir.AluOpType
AX = mybir.AxisListType

B, D = 16384, 1024
MEM = 100
NCORES = 8
ROWS = B // NCORES            # 2048 rows per core
NT = 512                      # rows per row-tile
NTILES = ROWS // NT           # 4
NSUB = ROWS // 128            # 16 row-subblocks per core
KD = D // 128                 # 8 feature chunks

MAX_WAITS = 1


def _split_excess_waits(nc):
    # walrus CTRL encoding caps sync waits per instruction; the TileContext
    # tail drain can exceed that. Move excess waits onto preceding NoOps.
    for fn in nc.m.functions:
        for bb in fn.blocks:
            if not isinstance(bb, mybir.BasicBlock):
                continue
            insts = bb.instructions
            i = 0
            while i < len(insts):
                ins = insts[i]
                si = getattr(ins, 'sync_info', None)
                waits = list(si.on_wait) if si is not None and si.on_wait else []
                if len(waits) > MAX_WAITS:
                    chunks = [waits[j:j + MAX_WAITS]
                              for j in range(0, len(waits), MAX_WAITS)]
                    si.on_wait = chunks[-1]
                    new_ops = [
                        mybir.InstNoOp(
                            name=f"{ins.name}-waitsplit-{k}",
                            engine=ins.engine,
                            sync_info=mybir.SyncInfo(on_wait=ch, on_update=[]),
                            bass_nofuse=True,
                        )
                        for k, ch in enumerate(chunks[:-1])
                    ]
                    insts[i:i] = new_ops
                    i += len(new_ops)
                i += 1


def build_kernel(reps: int = 1):
    nc = bass.Bass()

    pt_d = nc.dram_tensor("pt", [D, ROWS], BF16, kind="ExternalInput")
    at_d = nc.dram_tensor("at", [D, ROWS], BF16, kind="ExternalInput")
    prm_d = nc.dram_tensor("prm", [128, NSUB, D], BF16, kind="ExternalInput")
    arm_d = nc.dram_tensor("arm", [128, NSUB, D], BF16, kind="ExternalInput")
    wd_d = nc.dram_tensor("wd", [2 * D, D], BF16, kind="ExternalInput")
    wu_d = nc.dram_tensor("wu", [D, D // 2], BF16, kind="ExternalInput")
    wn_d = nc.dram_tensor("wn", [D, D // 2], BF16, kind="ExternalInput")
    wc1_d = nc.dram_tensor("wc1", [D, D // 4], BF16, kind="ExternalInput")
    wc2_d = nc.dram_tensor("wc2", [D // 4, D], BF16, kind="ExternalInput")
    wd2_d = nc.dram_tensor("wd2", [D, 1], BF16, kind="ExternalInput")
    wu2_d = nc.dram_tensor("wu2", [D // 2, 1], BF16, kind="ExternalInput")
    wn2_d = nc.dram_tensor("wn2", [D // 2, 1], BF16, kind="ExternalInput")
    mh_d = nc.dram_tensor("mh", [D, MEM], BF16, kind="ExternalInput")
    ones_d = nc.dram_tensor("ones", [128, 1], BF16, kind="ExternalInput")
    ident_d = nc.dram_tensor("ident", [128, 128], F32, kind="ExternalInput")
    bd1_d = nc.dram_tensor("bd1", [128, KD], F32, kind="ExternalInput")
    bu1_d = nc.dram_tensor("bu1", [128, 4], F32, kind="ExternalInput")
    bn1_d = nc.dram_tensor("bn1", [128, 4], F32, kind="ExternalInput")
    bc1_d = nc.dram_tensor("bc1", [128, 2], F32, kind="ExternalInput")
    bc2_d = nc.dram_tensor("bc2", [128, KD], F32, kind="ExternalInput")
    bh_d = nc.dram_tensor("bh", [128, 3], F32, kind="ExternalInput")  # d2,u2,n2
    out_d = nc.dram_tensor("out", [4, ROWS], F32, kind="ExternalOutput")

    with tile.TileContext(nc) as tc:
        import contextlib
        ctx = contextlib.ExitStack()
        with ctx:
            W = ctx.enter_context(tc.tile_pool(name="weights", bufs=1))
            io = ctx.enter_context(tc.tile_pool(name="io", bufs=2))
            big = ctx.enter_context(tc.tile_pool(name="big", bufs=2))
            sm = ctx.enter_context(tc.tile_pool(name="sm", bufs=2))
            acc = ctx.enter_context(tc.tile_pool(name="acc", bufs=1))
            mm = ctx.enter_context(tc.tile_pool(name="mmp", bufs=3, space="PSUM"))
            vec = ctx.enter_context(tc.tile_pool(name="vecp", bufs=2, space="PSUM"))
            simp = ctx.enter_context(tc.tile_pool(name="simp", bufs=1, space="PSUM"))
            trp = ctx.enter_context(tc.tile_pool(name="trp", bufs=1, space="PSUM"))

            # --- resident weights; emission order = DMA start order, so the
            # tensors needed by the first PE work go first and the big
            # dissonance weight streams in under early compute.
            mh = W.tile([128, KD, MEM], BF16)
            nc.sync.dma_start(mh, mh_d.rearrange("(k p) m -> p k m", p=128))

            # tile-0 inputs next: sims/entropy/small MLPs can start while the
            # remaining 7 MiB of weights stream in
            def load_io(t):
                rs = slice(t * NT, (t + 1) * NT)
                at = io.tile([128, KD, NT], BF16, tag="at")
                nc.sync.dma_start(
                    at, at_d[:, rs].rearrange("(k p) r -> p k r", p=128))
                pt = io.tile([128, KD, NT], BF16, tag="pt")
                nc.sync.dma_start(
                    pt, pt_d[:, rs].rearrange("(k p) r -> p k r", p=128))
                prm = io.tile([128, 4, D], BF16, tag="prm")
                nc.sync.dma_start(prm, prm_d[:, 4 * t:4 * t + 4, :])
                arm = io.tile([128, 4, D], BF16, tag="arm")
                nc.sync.dma_start(arm, arm_d[:, 4 * t:4 * t + 4, :])
                return at, pt, prm, arm

            io0 = load_io(0)

            wu = W.tile([128, KD, D // 2], BF16)
            nc.sync.dma_start(wu, wu_d.rearrange("(k p) m -> p k m", p=128))
            wn = W.tile([128, KD, D // 2], BF16)
            nc.sync.dma_start(wn, wn_d.rearrange("(k p) m -> p k m", p=128))
            wc1 = W.tile([128, KD, D // 4], BF16)
            nc.sync.dma_start(wc1, wc1_d.rearrange("(k p) m -> p k m", p=128))
            wc2 = W.tile([128, 2, D], BF16)
            nc.sync.dma_start(wc2, wc2_d.rearrange("(k p) m -> p k m", p=128))
            wd2 = W.tile([128, KD, 1], BF16)
            nc.sync.dma_start(wd2, wd2_d.rearrange("(k p) m -> p k m", p=128))
            wu2 = W.tile([128, 4, 1], BF16)
            nc.sync.dma_start(wu2, wu2_d.rearrange("(k p) m -> p k m", p=128))
            wn2 = W.tile([128, 4, 1], BF16)
            nc.sync.dma_start(wn2, wn2_d.rearrange("(k p) m -> p k m", p=128))
            ones = W.tile([128, 1], BF16)
            nc.sync.dma_start(ones, ones_d[:])
            ident = W.tile([128, 128], F32)
            nc.sync.dma_start(ident, ident_d[:])
            bd1 = W.tile([128, KD], F32)
            nc.sync.dma_start(bd1, bd1_d[:])
            bu1 = W.tile([128, 4], F32)
            nc.sync.dma_start(bu1, bu1_d[:])
            bn1 = W.tile([128, 4], F32)
            nc.sync.dma_start(bn1, bn1_d[:])
            bc1 = W.tile([128, 2], F32)
            nc.sync.dma_start(bc1, bc1_d[:])
            bc2 = W.tile([128, KD], F32)
            nc.sync.dma_start(bc2, bc2_d[:])
            bh = W.tile([128, 3], F32)
            nc.sync.dma_start(bh, bh_d[:])
            # dissonance weight, split per k-chunk so each accumulation step
            # only waits for its own chunk
            wd = []
            for k in range(16):
                wk = W.tile([128, D], BF16, tag=f"wdc{k}")
                nc.sync.dma_start(wk, wd_d[k * 128:(k + 1) * 128, :])
                wd.append(wk)

            for _ in range(reps):
                # per-core row-major scalar accumulators [128 rows, 16 subs]
                zZ = acc.tile([128, NSUB], F32, tag="zZ")
                zS = acc.tile([128, NSUB], F32, tag="zS")
                na2 = acc.tile([128, NSUB], F32, tag="na2")
                raw = acc.tile([128, NSUB], F32, tag="raw")
                zAll = acc.tile([128, NTILES, 4, 4], F32, tag="zAll")

                for t in range(NTILES):
                    at, pt, prm, arm = io0 if t == 0 else load_io(t)

                    # --- sims (needs only mh + at): row-major max cos sim
                    for s in range(4):
                        g = 4 * t + s
                        pss = simp.tile([128, MEM], F32, tag="simp")
                        for k in range(KD):
                            nc.tensor.matmul(
                                pss, at[:, k, s * 128:(s + 1) * 128],
                                mh[:, k, :],
                                start=(k == 0), stop=(k == KD - 1))
                        nc.vector.reduce_max(raw[:, g:g + 1], pss, axis=AX.X)

                    # --- entropy sums + ||a||^2, row-major free-dim reduces
                    # fused into the elementwise DVE ops via accum_out
                    for s in range(4):
                        g = 4 * t + s
                        e = sm.tile([128, D], BF16, tag="e")
                        nc.scalar.activation(e, prm[:, s, :], AF.Exp)
                        e2 = sm.tile([128, D], BF16, tag="e2")
                        nc.vector.tensor_scalar(
                            e2, e, 1.0, 0.0, op0=ALU.mult, op1=ALU.add,
                            accum_out=zZ[:, g:g + 1])
                        nc.vector.scalar_tensor_tensor(
                            e2, e, 1.0, prm[:, s, :],
                            op0=ALU.mult, op1=ALU.mult,
                            accum_out=zS[:, g:g + 1])
                        nc.vector.scalar_tensor_tensor(
                            e2, arm[:, s, :], 1.0, arm[:, s, :],
                            op0=ALU.mult, op1=ALU.mult,
                            accum_out=na2[:, g:g + 1])

                    # per-row scalars staged at quadrant-aligned partitions
                    # {0,32,64,96} (engine writes must be 32-aligned)
                    z4 = sm.tile([128, NT], F32, tag="z4")

                    # --- uncertainty MLP
                    hu = big.tile([128, 4, NT], BF16, tag="hu")
                    for j in range(4):
                        ps = mm.tile([128, NT], F32, tag="mm")
                        for k in range(KD):
                            nc.tensor.matmul(ps, wu[:, k, j * 128:(j + 1) * 128],
                                             pt[:, k, :],
                                             start=(k == 0), stop=(k == KD - 1))
                        nc.scalar.activation(hu[:, j, :], ps, AF.Relu,
                                             bias=bu1[:, j:j + 1])
                    zu = vec.tile([1, NT], F32, tag="vec")
                    for j in range(4):
                        nc.tensor.matmul(zu, wu2[:, j, :], hu[:, j, :],
                                         start=(j == 0), stop=(j == 3))
                    nc.scalar.copy(z4[32:33, :], zu)

                    # --- novelty MLP
                    hn = big.tile([128, 4, NT], BF16, tag="hn")
                    for j in range(4):
                        ps = mm.tile([128, NT], F32, tag="mm")
                        for k in range(KD):
                            nc.tensor.matmul(ps, wn[:, k, j * 128:(j + 1) * 128],
                                             at[:, k, :],
                                             start=(k == 0), stop=(k == KD - 1))
                        nc.scalar.activation(hn[:, j, :], ps, AF.Relu,
                                             bias=bn1[:, j:j + 1])
                    zn = vec.tile([1, NT], F32, tag="vec")
                    for j in range(4):
                        nc.tensor.matmul(zn, wn2[:, j, :], hn[:, j, :],
                                         start=(j == 0), stop=(j == 3))
                    nc.scalar.copy(z4[64:65, :], zn)

                    # --- compression
                    hc = big.tile([128, 2, NT], BF16, tag="hc")
                    for j in range(2):
                        ps = mm.tile([128, NT], F32, tag="mm")
                        for k in range(KD):
                            nc.tensor.matmul(ps, wc1[:, k, j * 128:(j + 1) * 128],
                                             pt[:, k, :],
                                             start=(k == 0), stop=(k == KD - 1))
                        nc.scalar.activation(hc[:, j, :], ps, AF.Relu,
                                             bias=bc1[:, j:j + 1])
                    ms = vec.tile([1, NT], F32, tag="vec")
                    for j in range(KD):
                        psr = mm.tile([128, NT], F32, tag="mm")
                        for k in range(2):
                            nc.tensor.matmul(psr, wc2[:, k, j * 128:(j + 1) * 128],
                                             hc[:, k, :],
                                             start=(k == 0), stop=(k == 1))
                        dj = sm.tile([128, NT], BF16, tag="dj")
                        # dj = (recon + bc2) - p  (sign-flipped diff; squared)
                        nc.vector.scalar_tensor_tensor(
                            dj, psr, bc2[:, j:j + 1], pt[:, j, :],
                            op0=ALU.add, op1=ALU.subtract)
                        dsq = sm.tile([128, NT], BF16, tag="dsq")
                        nc.scalar.activation(dsq, dj, AF.Square)
                        nc.tensor.matmul(ms, ones, dsq,
                                         start=(j == 0), stop=(j == KD - 1))
                    nc.vector.tensor_copy(z4[96:97, :], ms)

                    # --- dissonance
                    hd = big.tile([128, KD, NT], BF16, tag="hd")
                    for j in range(KD):
                        ps = mm.tile([128, NT], F32, tag="mm")
                        for k in range(16):
                            x = pt[:, k, :] if k < KD else at[:, k - KD, :]
                            nc.tensor.matmul(ps, wd[k][:, j * 128:(j + 1) * 128],
                                             x, start=(k == 0), stop=(k == 15))
                        nc.scalar.activation(hd[:, j, :], ps, AF.Relu,
                                             bias=bd1[:, j:j + 1])
                    zd = vec.tile([1, NT], F32, tag="vec")
                    for j in range(KD):
                        nc.tensor.matmul(zd, wd2[:, j, :], hd[:, j, :],
                                         start=(j == 0), stop=(j == KD - 1))
                    nc.scalar.copy(z4[0:1, :], zd)

                    # --- transpose the 4 per-row scalars into row-major.
                    # Full [128,128] transposes (K=128 is the only shape the
                    # backend handles); quantities land on cols {0,32,64,96},
                    # picked out by a strided DVE copy per subblock.
                    for s in range(4):
                        ztr = trp.tile([128, 4, 32], F32, tag="ztr")
                        nc.tensor.transpose(
                            ztr, z4[:, s * 128:(s + 1) * 128], ident)
                        nc.vector.tensor_copy(zAll[:, t, s, :], ztr[:, :, 0])

                # --- per-core finishing, all [128 rows, 16 subs]
                zdR = zAll[:, :, :, 0]
                zuR = zAll[:, :, :, 1]
                znR = zAll[:, :, :, 2]
                msR = zAll[:, :, :, 3]

                fin = acc.tile([128, 12, NSUB], F32, tag="fin")
                spD, spU, spN = fin[:, 0, :], fin[:, 1, :], fin[:, 2, :]
                tmp1, tmp2, tmp3 = fin[:, 3, :], fin[:, 4, :], fin[:, 5, :]
                uncR, novR, compR = fin[:, 6, :], fin[:, 7, :], fin[:, 8, :]

                # softplus(z + b) = Ln(1 + Exp(z + b))
                nc.scalar.activation(tmp1, zdR, AF.Exp, bias=bh[:, 0:1])
                nc.scalar.activation(spD, tmp1, AF.Ln, bias=1.0)
                nc.scalar.activation(tmp1, zuR, AF.Exp, bias=bh[:, 1:2])
                nc.scalar.activation(spU, tmp1, AF.Ln, bias=1.0)
                nc.scalar.activation(tmp1, znR, AF.Exp, bias=bh[:, 2:3])
                nc.scalar.activation(spN, tmp1, AF.Ln, bias=1.0)

                # uncertainty = spU + 0.1 * (lnZ - zS/Z)
                nc.scalar.activation(tmp1, zZ, AF.Ln)          # lnZ
                nc.vector.reciprocal(tmp2, zZ)                 # 1/Z
                nc.vector.tensor_mul(tmp3, zS, tmp2)           # zS/Z
                nc.vector.tensor_tensor(tmp1, tmp1, tmp3, ALU.subtract)
                nc.vector.scalar_tensor_tensor(
                    uncR, tmp1, 0.1, spU, op0=ALU.mult, op1=ALU.add)

                # novelty = 0.7*(1 - raw/||a||) + 0.3*spN
                nc.scalar.activation(tmp1, na2, AF.Ln)
                nc.scalar.activation(tmp2, tmp1, AF.Exp, scale=-0.5)  # 1/||a||
                nc.vector.tensor_mul(tmp3, raw, tmp2)          # cos
                nc.vector.tensor_scalar_mul(tmp1, spN, 0.3)
                nc.vector.scalar_tensor_tensor(
                    novR, tmp3, -0.7, tmp1, op0=ALU.mult, op1=ALU.add)
                nc.vector.tensor_scalar_add(novR, novR, 0.7)

                nc.vector.tensor_scalar_mul(compR, msR, 1.0 / D)

                # --- assemble [4, ROWS] output: transpose each signal
                for q, src in enumerate((spD, uncR, novR, compR)):
                    oT = trp.tile([NSUB, 128], F32, tag="oT")
                    nc.tensor.transpose(oT, src, ident)
                    ob = sm.tile([NSUB, 128], F32, tag="ob")
                    nc.scalar.copy(ob, oT)
                    nc.sync.dma_start(
                        out_d[q:q + 1, :].rearrange("a (s r) -> (a s) r", s=NSUB),
                        ob)

    _split_excess_waits(nc)
    return nc


def _prep_inputs(prediction, actual, pattern_memory,
                 W_d1, b_d1, W_d2, b_d2, W_u1, b_u1, W_u2, b_u2,
                 W_n1, b_n1, W_n2, b_n2, W_c1, b_c1, W_c2, b_c2):
    bf = ml_dtypes.bfloat16

    def t_bf(a):  # transposed contiguous bf16
        return np.ascontiguousarray(np.asarray(a, np.float32).T).astype(bf)

    mnorm = np.maximum(np.linalg.norm(
        np.asarray(pattern_memory, np.float32), axis=1), 1e-8)
    mhat = np.asarray(pattern_memory, np.float32) / mnorm[:, None]

    def fold_bias(b, chunks):
        return np.ascontiguousarray(
            np.asarray(b, np.float32).reshape(chunks, 128).T)

    bh = np.empty((128, 3), np.float32)
    bh[:, 0] = float(np.asarray(b_d2).reshape(-1)[0])
    bh[:, 1] = float(np.asarray(b_u2).reshape(-1)[0])
    bh[:, 2] = float(np.asarray(b_n2).reshape(-1)[0])

    shared = {
        "wd": t_bf(W_d1), "wu": t_bf(W_u1), "wn": t_bf(W_n1),
        "wc1": t_bf(W_c1), "wc2": t_bf(W_c2),
        "wd2": t_bf(W_d2), "wu2": t_bf(W_u2), "wn2": t_bf(W_n2),
        "mh": t_bf(mhat),
        "ones": np.ones((128, 1), bf),
        "ident": np.eye(128, dtype=np.float32),
        "bd1": fold_bias(b_d1, KD), "bu1": fold_bias(b_u1, 4),
        "bn1": fold_bias(b_n1, 4), "bc1": fold_bias(b_c1, 2),
        "bc2": fold_bias(b_c2, KD),
        "bh": bh,
    }
    p32 = np.asarray(prediction, np.float32)
    a32 = np.asarray(actual, np.float32)
    in_maps = []
    for c in range(NCORES):
        rows = slice(c * ROWS, (c + 1) * ROWS)
        m = dict(shared)
        m["pt"] = np.ascontiguousarray(p32[rows].T).astype(bf)
        m["at"] = np.ascontiguousarray(a32[rows].T).astype(bf)
        m["prm"] = np.ascontiguousarray(
            p32[rows].reshape(NSUB, 128, D).transpose(1, 0, 2)).astype(bf)
        m["arm"] = np.ascontiguousarray(
            a32[rows].reshape(NSUB, 128, D).transpose(1, 0, 2)).astype(bf)
        in_maps.append(m)
    return in_maps


_NC_CACHE = {}


def kernel(**inputs) -> np.ndarray:
    in_maps = _prep_inputs(**inputs)
    if 'nc' not in _NC_CACHE:
        _NC_CACHE['nc'] = build_kernel(reps=1)
    nc = _NC_CACHE['nc']
    res = run_bass_kernel_spmd(nc, in_maps, core_ids=list(range(NCORES)))
    out = np.empty((B, 4), np.float32)
    for c in range(NCORES):
        out[c * ROWS:(c + 1) * ROWS, :] = res.results[c]["out"].T
    return out


# revision 27
# speedup vs baseline: 1.1520x; 1.0473x over previous
"""Trainium2 Bass kernel for IntrinsicSignalSynthesizer.

Data-parallel over 8 NeuronCores: batch 16384 -> 8 x 2048 rows.

Design notes (v5):
- MLP matmuls are feature-major ([128 feat, kchunk, rows] tiles): every
  matmul contracts over the partition dim at the bf16 stream roofline.
- Per-row reductions (sum e^p, sum p*e^p, sum a^2) run OFF the PE: the
  inputs are DMA'd a second time in row-major layout and reduced along
  the free dim with DVE tensor_scalar/scalar_tensor_tensor accum_out.
- Per-row scalar finishing is row-major [128 rows, 4 subblocks] per
  row-tile; the PE-produced [1, 512] scalars (3 MLP heads + compression
  sum) are staged at quadrant partitions {0,32,64,96} of one tile and
  moved row-major with a single [128,128] PE transpose per subblock.
- DMA order: pattern memory + tile-0 feature-major inputs first, then
  small weights, then the 4 MiB dissonance weight in 4 chunks - the PE
  starts on sims/small MLPs ~4 us in while the big weight streams.
- Finishing is per-row-tile so the epilogue of the last tile is short.
"""
import sys
sys.path.insert(0, '/opt/trn_rl_repo')

import numpy as np
import ml_dtypes

import concourse.bass as bass
import concourse.mybir as mybir
import concourse.tile as tile
from concourse.bass_utils import run_bass_kernel_spmd

BF16 = mybir.dt.bfloat16
F32 = mybir.dt.float32
AF = mybir.ActivationFunctionType
ALU = mybir.AluOpType
AX = mybir.AxisListType

B, D = 16384, 1024
MEM = 100
NCORES = 8
ROWS = B // NCORES            # 2048 rows per core
NT = 512                      # rows per row-tile
NTILES = ROWS // NT           # 4
NSUB = ROWS // 128            # 16 row-subblocks per core
KD = D // 128                 # 8 feature chunks

MAX_WAITS = 1


def _split_excess_waits(nc):
    # walrus CTRL encoding caps sync waits per instruction; the TileContext
    # tail drain can exceed that. Move excess waits onto preceding NoOps.
    for fn in nc.m.functions:
        for bb in fn.blocks:
            if not isinstance(bb, mybir.BasicBlock):
                continue
            insts = bb.instructions
            i = 0
            while i < len(insts):
                ins = insts[i]
                si = getattr(ins, 'sync_info', None)
                waits = list(si.on_wait) if si is not None and si.on_wait else []
                if len(waits) > MAX_WAITS:
                    chunks = [waits[j:j + MAX_WAITS]
                              for j in range(0, len(waits), MAX_WAITS)]
                    si.on_wait = chunks[-1]
                    new_ops = [
                        mybir.InstNoOp(
                            name=f"{ins.name}-waitsplit-{k}",
                            engine=ins.engine,
                            sync_info=mybir.SyncInfo(on_wait=ch, on_update=[]),
                            bass_nofuse=True,
                        )
                        for k, ch in enumerate(chunks[:-1])
                    ]
                    insts[i:i] = new_ops
                    i += len(new_ops)
                i += 1


def build_kernel(reps: int = 1):
    assert reps == 1, "tile-0 input prefetch assumes a single pass"
    nc = bass.Bass()

    pt_d = nc.dram_tensor("pt", [D, ROWS], BF16, kind="ExternalInput")
    at_d = nc.dram_tensor("at", [D, ROWS], BF16, kind="ExternalInput")
    prm_d = nc.dram_tensor("prm", [128, NSUB, D], BF16, kind="ExternalInput")
    arm_d = nc.dram_tensor("arm", [128, NSUB, D], BF16, kind="ExternalInput")
    wd_d = nc.dram_tensor("wd", [2 * D, D], BF16, kind="ExternalInput")
    wu_d = nc.dram_tensor("wu", [D, D // 2], BF16, kind="ExternalInput")
    wn_d = nc.dram_tensor("wn", [D, D // 2], BF16, kind="ExternalInput")
    wc1_d = nc.dram_tensor("wc1", [D, D // 4], BF16, kind="ExternalInput")
    wc2_d = nc.dram_tensor("wc2", [D // 4, D], BF16, kind="ExternalInput")
    wd2_d = nc.dram_tensor("wd2", [D, 1], BF16, kind="ExternalInput")
    wu2_d = nc.dram_tensor("wu2", [D // 2, 1], BF16, kind="ExternalInput")
    wn2_d = nc.dram_tensor("wn2", [D // 2, 1], BF16, kind="ExternalInput")
    mh_d = nc.dram_tensor("mh", [D, MEM], BF16, kind="ExternalInput")
    ones_d = nc.dram_tensor("ones", [128, 1], BF16, kind="ExternalInput")
    ident_d = nc.dram_tensor("ident", [128, 128], F32, kind="ExternalInput")
    bd1_d = nc.dram_tensor("bd1", [128, KD], F32, kind="ExternalInput")
    bu1_d = nc.dram_tensor("bu1", [128, 4], F32, kind="ExternalInput")
    bn1_d = nc.dram_tensor("bn1", [128, 4], F32, kind="ExternalInput")
    bc1_d = nc.dram_tensor("bc1", [128, 2], F32, kind="ExternalInput")
    bc2_d = nc.dram_tensor("bc2", [128, KD], F32, kind="ExternalInput")
    bh_d = nc.dram_tensor("bh", [128, 3], F32, kind="ExternalInput")  # d2,u2,n2
    out_d = nc.dram_tensor("out", [4, ROWS], F32, kind="ExternalOutput")

    with tile.TileContext(nc) as tc:
        import contextlib
        ctx = contextlib.ExitStack()
        with ctx:
            W = ctx.enter_context(tc.tile_pool(name="weights", bufs=1))
            io = ctx.enter_context(tc.tile_pool(name="io", bufs=3))
            io2 = ctx.enter_context(tc.tile_pool(name="io2", bufs=2))
            big = ctx.enter_context(tc.tile_pool(name="big", bufs=2))
            sm = ctx.enter_context(tc.tile_pool(name="sm", bufs=2))
            mm = ctx.enter_context(tc.tile_pool(name="mmp", bufs=3, space="PSUM"))
            vec = ctx.enter_context(tc.tile_pool(name="vecp", bufs=2, space="PSUM"))
            simp = ctx.enter_context(tc.tile_pool(name="simp", bufs=1, space="PSUM"))
            trp = ctx.enter_context(tc.tile_pool(name="trp", bufs=1, space="PSUM"))

            # --- DMA emission order == start order. First what the PE needs
            # first: pattern memory, tile-0 FM inputs, small weights; the big
            # dissonance weight last, streaming under early compute.
            mh = W.tile([128, KD, MEM], BF16)
            nc.sync.dma_start(mh, mh_d.rearrange("(k p) m -> p k m", p=128))

            def load_fm(t):
                rs = slice(t * NT, (t + 1) * NT)
                at = io.tile([128, KD, NT], BF16, tag="at")
                nc.sync.dma_start(
                    at, at_d[:, rs].rearrange("(k p) r -> p k r", p=128))
                pt = io.tile([128, KD, NT], BF16, tag="pt")
                nc.sync.dma_start(
                    pt, pt_d[:, rs].rearrange("(k p) r -> p k r", p=128))
                return at, pt

            def load_rm(t):
                prm = io2.tile([128, 4, D], BF16, tag="prm")
                nc.sync.dma_start(prm, prm_d[:, 4 * t:4 * t + 4, :])
                arm = io2.tile([128, 4, D], BF16, tag="arm")
                nc.sync.dma_start(arm, arm_d[:, 4 * t:4 * t + 4, :])
                return prm, arm

            fm0 = load_fm(0)

            wu = W.tile([128, KD, D // 2], BF16)
            nc.sync.dma_start(wu, wu_d.rearrange("(k p) m -> p k m", p=128))
            wn = W.tile([128, KD, D // 2], BF16)
            nc.sync.dma_start(wn, wn_d.rearrange("(k p) m -> p k m", p=128))
            wc1 = W.tile([128, KD, D // 4], BF16)
            nc.sync.dma_start(wc1, wc1_d.rearrange("(k p) m -> p k m", p=128))
            wc2 = W.tile([128, 2, D], BF16)
            nc.sync.dma_start(wc2, wc2_d.rearrange("(k p) m -> p k m", p=128))
            wd2 = W.tile([128, KD, 1], BF16)
            nc.sync.dma_start(wd2, wd2_d.rearrange("(k p) m -> p k m", p=128))
            wu2 = W.tile([128, 4, 1], BF16)
            nc.sync.dma_start(wu2, wu2_d.rearrange("(k p) m -> p k m", p=128))
            wn2 = W.tile([128, 4, 1], BF16)
            nc.sync.dma_start(wn2, wn2_d.rearrange("(k p) m -> p k m", p=128))
            ones = W.tile([128, 1], BF16)
            nc.sync.dma_start(ones, ones_d[:])
            ident = W.tile([128, 128], F32)
            nc.sync.dma_start(ident, ident_d[:])
            bd1 = W.tile([128, KD], F32)
            nc.sync.dma_start(bd1, bd1_d[:])
            bu1 = W.tile([128, 4], F32)
            nc.sync.dma_start(bu1, bu1_d[:])
            bn1 = W.tile([128, 4], F32)
            nc.sync.dma_start(bn1, bn1_d[:])
            bc1 = W.tile([128, 2], F32)
            nc.sync.dma_start(bc1, bc1_d[:])
            bc2 = W.tile([128, KD], F32)
            nc.sync.dma_start(bc2, bc2_d[:])
            bh = W.tile([128, 3], F32)
            nc.sync.dma_start(bh, bh_d[:])
            rm0 = load_rm(0)
            # dissonance weight in 4 chunk-group DMAs
            wd = []
            for g in range(4):
                wg = W.tile([128, 4, D], BF16, tag=f"wdc{g}")
                nc.sync.dma_start(
                    wg, wd_d[g * 512:(g + 1) * 512, :].rearrange(
                        "(k p) m -> p k m", p=128))
                wd.append(wg)

            def wdk(k):
                return wd[k // 4][:, k % 4, :]

            for t in range(NTILES):
                at, pt = fm0 if t == 0 else load_fm(t)
                prm, arm = rm0 if t == 0 else load_rm(t)
                raw = sm.tile([128, 4], F32, tag="raw")
                zZ = sm.tile([128, 4], F32, tag="zZ")
                zS = sm.tile([128, 4], F32, tag="zS")
                na2 = sm.tile([128, 4], F32, tag="na2")

                # --- sims (needs only mh + at): row-major max cos sim
                for s in range(4):
                    pss = simp.tile([128, MEM], F32, tag="simp")
                    for k in range(KD):
                        nc.tensor.matmul(
                            pss, at[:, k, s * 128:(s + 1) * 128],
                            mh[:, k, :],
                            start=(k == 0), stop=(k == KD - 1))
                    nc.vector.reduce_max(raw[:, s:s + 1], pss, axis=AX.X)

                # per-row scalars staged at quadrant-aligned partitions
                # {0,32,64,96} (engine writes must be 32-aligned)
                z4 = sm.tile([128, NT], F32, tag="z4")

                # --- uncertainty MLP
                hu = big.tile([128, 4, NT], BF16, tag="hu")
                for j in range(4):
                    ps = mm.tile([128, NT], F32, tag="mm")
                    for k in range(KD):
                        nc.tensor.matmul(ps, wu[:, k, j * 128:(j + 1) * 128],
                                         pt[:, k, :],
                                         start=(k == 0), stop=(k == KD - 1))
                    nc.scalar.activation(hu[:, j, :], ps, AF.Relu,
                                         bias=bu1[:, j:j + 1])
                zu = vec.tile([1, NT], F32, tag="vec")
                for j in range(4):
                    nc.tensor.matmul(zu, wu2[:, j, :], hu[:, j, :],
                                     start=(j == 0), stop=(j == 3))
                nc.scalar.copy(z4[32:33, :], zu)

                # --- entropy sums + ||a||^2: row-major free-dim reduces
                # fused into DVE elementwise ops via accum_out
                for s in range(4):
                    e = sm.tile([128, D], BF16, tag="e")
                    nc.scalar.activation(e, prm[:, s, :], AF.Exp)
                    e2 = sm.tile([128, D], BF16, tag="e2")
                    nc.vector.tensor_scalar(
                        e2, e, 1.0, 0.0, op0=ALU.mult, op1=ALU.add,
                        accum_out=zZ[:, s:s + 1])
                    nc.vector.scalar_tensor_tensor(
                        e2, e, 1.0, prm[:, s, :],
                        op0=ALU.mult, op1=ALU.mult,
                        accum_out=zS[:, s:s + 1])
                    nc.vector.scalar_tensor_tensor(
                        e2, arm[:, s, :], 1.0, arm[:, s, :],
                        op0=ALU.mult, op1=ALU.mult,
                        accum_out=na2[:, s:s + 1])

                # --- novelty MLP
                hn = big.tile([128, 4, NT], BF16, tag="hn")
                for j in range(4):
                    ps = mm.tile([128, NT], F32, tag="mm")
                    for k in range(KD):
                        nc.tensor.matmul(ps, wn[:, k, j * 128:(j + 1) * 128],
                                         at[:, k, :],
                                         start=(k == 0), stop=(k == KD - 1))
                    nc.scalar.activation(hn[:, j, :], ps, AF.Relu,
                                         bias=bn1[:, j:j + 1])
                zn = vec.tile([1, NT], F32, tag="vec")
                for j in range(4):
                    nc.tensor.matmul(zn, wn2[:, j, :], hn[:, j, :],
                                     start=(j == 0), stop=(j == 3))
                nc.scalar.copy(z4[64:65, :], zn)

                # --- compression
                hc = big.tile([128, 2, NT], BF16, tag="hc")
                for j in range(2):
                    ps = mm.tile([128, NT], F32, tag="mm")
                    for k in range(KD):
                        nc.tensor.matmul(ps, wc1[:, k, j * 128:(j + 1) * 128],
                                         pt[:, k, :],
                                         start=(k == 0), stop=(k == KD - 1))
                    nc.scalar.activation(hc[:, j, :], ps, AF.Relu,
                                         bias=bc1[:, j:j + 1])
                ms = vec.tile([1, NT], F32, tag="vec")
                for j in range(KD):
                    psr = mm.tile([128, NT], F32, tag="mm")
                    for k in range(2):
                        nc.tensor.matmul(psr, wc2[:, k, j * 128:(j + 1) * 128],
                                         hc[:, k, :],
                                         start=(k == 0), stop=(k == 1))
                    dj = sm.tile([128, NT], BF16, tag="dj")
                    # dj = (recon + bc2) - p  (sign-flipped diff; squared)
                    nc.vector.scalar_tensor_tensor(
                        dj, psr, bc2[:, j:j + 1], pt[:, j, :],
                        op0=ALU.add, op1=ALU.subtract)
                    dsq = sm.tile([128, NT], BF16, tag="dsq")
                    nc.scalar.activation(dsq, dj, AF.Square)
                    nc.tensor.matmul(ms, ones, dsq,
                                     start=(j == 0), stop=(j == KD - 1))
                nc.vector.tensor_copy(z4[96:97, :], ms)

                # --- dissonance
                hd = big.tile([128, KD, NT], BF16, tag="hd")
                for j in range(KD):
                    ps = mm.tile([128, NT], F32, tag="mm")
                    for k in range(16):
                        x = pt[:, k, :] if k < KD else at[:, k - KD, :]
                        nc.tensor.matmul(ps, wdk(k)[:, j * 128:(j + 1) * 128],
                                         x, start=(k == 0), stop=(k == 15))
                    nc.scalar.activation(hd[:, j, :], ps, AF.Relu,
                                         bias=bd1[:, j:j + 1])
                zd = vec.tile([1, NT], F32, tag="vec")
                for j in range(KD):
                    nc.tensor.matmul(zd, wd2[:, j, :], hd[:, j, :],
                                     start=(j == 0), stop=(j == KD - 1))
                nc.scalar.copy(z4[0:1, :], zd)

                # --- move the 4 per-row scalars row-major: one [128,128]
                # PE transpose per subblock; quantities land on cols
                # {0,32,64,96} (zd, zu, zn, ms).
                zAll = sm.tile([128, 4, 4], F32, tag="zAll")
                for s in range(4):
                    ztr = trp.tile([128, 4, 32], F32, tag="ztr")
                    nc.tensor.transpose(
                        ztr, z4[:, s * 128:(s + 1) * 128], ident)
                    nc.vector.tensor_copy(zAll[:, s, :], ztr[:, :, 0])

                # --- per-tile finishing, row-major [128 rows, 4 subs]
                zdR = zAll[:, :, 0]
                zuR = zAll[:, :, 1]
                znR = zAll[:, :, 2]
                msR = zAll[:, :, 3]
                fin = sm.tile([128, 12, 4], F32, tag="fin")
                spD, spU, spN = fin[:, 0, :], fin[:, 1, :], fin[:, 2, :]
                tmp1, tmp2, tmp3 = fin[:, 3, :], fin[:, 4, :], fin[:, 5, :]
                uncR, novR = fin[:, 6, :], fin[:, 7, :]
                tmp4, tmp5 = fin[:, 8, :], fin[:, 9, :]

                # softplus(z + b) = Ln(1 + Exp(z + b))
                nc.scalar.activation(tmp1, zdR, AF.Exp, bias=bh[:, 0:1])
                nc.scalar.activation(spD, tmp1, AF.Ln, bias=1.0)
                nc.scalar.activation(tmp2, zuR, AF.Exp, bias=bh[:, 1:2])
                nc.scalar.activation(spU, tmp2, AF.Ln, bias=1.0)
                nc.scalar.activation(tmp3, znR, AF.Exp, bias=bh[:, 2:3])
                nc.scalar.activation(spN, tmp3, AF.Ln, bias=1.0)

                # uncertainty = spU + 0.1 * (lnZ - zS/Z)
                nc.scalar.activation(tmp1, zZ, AF.Ln)      # lnZ
                nc.vector.reciprocal(tmp2, zZ)             # 1/Z
                nc.vector.tensor_mul(tmp3, zS, tmp2)       # zS/Z
                nc.vector.tensor_tensor(tmp1, tmp1, tmp3, ALU.subtract)
                nc.vector.scalar_tensor_tensor(
                    uncR, tmp1, 0.1, spU, op0=ALU.mult, op1=ALU.add)

                # novelty = 0.7*(1 - raw/||a||) + 0.3*spN
                nc.scalar.activation(tmp4, na2, AF.Ln)
                nc.scalar.activation(tmp5, tmp4, AF.Exp, scale=-0.5)  # 1/||a||
                nc.vector.tensor_mul(tmp4, raw, tmp5)      # cos
                nc.vector.tensor_scalar_mul(tmp5, spN, 0.3)
                nc.vector.scalar_tensor_tensor(
                    novR, tmp4, -0.7, tmp5, op0=ALU.mult, op1=ALU.add)
                nc.vector.tensor_scalar_add(novR, novR, 0.7)

                nc.vector.tensor_scalar_mul(tmp5, msR, 1.0 / D)     # comp

                # --- assemble this tile's [4, NT] output slice
                rs = slice(t * NT, (t + 1) * NT)
                for q, src in enumerate((spD, uncR, novR, tmp5)):
                    oT = trp.tile([4, 128], F32, tag="oT")
                    nc.tensor.transpose(oT, src, ident)
                    ob = sm.tile([4, 128], F32, tag="ob")
                    nc.scalar.copy(ob, oT)
                    nc.sync.dma_start(
                        out_d[q:q + 1, rs].rearrange(
                            "a (s r) -> (a s) r", s=4),
                        ob)

    _split_excess_waits(nc)
    return nc


def _prep_inputs(prediction, actual, pattern_memory,
                 W_d1, b_d1, W_d2, b_d2, W_u1, b_u1, W_u2, b_u2,
                 W_n1, b_n1, W_n2, b_n2, W_c1, b_c1, W_c2, b_c2):
    bf = ml_dtypes.bfloat16

    def t_bf(a):  # transposed contiguous bf16
        return np.ascontiguousarray(np.asarray(a, np.float32).T).astype(bf)

    mnorm = np.maximum(np.linalg.norm(
        np.asarray(pattern_memory, np.float32), axis=1), 1e-8)
    mhat = np.asarray(pattern_memory, np.float32) / mnorm[:, None]

    def fold_bias(b, chunks):
        return np.ascontiguousarray(
            np.asarray(b, np.float32).reshape(chunks, 128).T)

    bh = np.empty((128, 3), np.float32)
    bh[:, 0] = float(np.asarray(b_d2).reshape(-1)[0])
    bh[:, 1] = float(np.asarray(b_u2).reshape(-1)[0])
    bh[:, 2] = float(np.asarray(b_n2).reshape(-1)[0])

    shared = {
        "wd": t_bf(W_d1), "wu": t_bf(W_u1), "wn": t_bf(W_n1),
        "wc1": t_bf(W_c1), "wc2": t_bf(W_c2),
        "wd2": t_bf(W_d2), "wu2": t_bf(W_u2), "wn2": t_bf(W_n2),
        "mh": t_bf(mhat),
        "ones": np.ones((128, 1), bf),
        "ident": np.eye(128, dtype=np.float32),
        "bd1": fold_bias(b_d1, KD), "bu1": fold_bias(b_u1, 4),
        "bn1": fold_bias(b_n1, 4), "bc1": fold_bias(b_c1, 2),
        "bc2": fold_bias(b_c2, KD),
        "bh": bh,
    }
    p32 = np.asarray(prediction, np.float32)
    a32 = np.asarray(actual, np.float32)
    in_maps = []
    for c in range(NCORES):
        rows = slice(c * ROWS, (c + 1) * ROWS)
        m = dict(shared)
        m["pt"] = np.ascontiguousarray(p32[rows].T).astype(bf)
        m["at"] = np.ascontiguousarray(a32[rows].T).astype(bf)
        m["prm"] = np.ascontiguousarray(
            p32[rows].reshape(NSUB, 128, D).transpose(1, 0, 2)).astype(bf)
        m["arm"] = np.ascontiguousarray(
            a32[rows].reshape(NSUB, 128, D).transpose(1, 0, 2)).astype(bf)
        in_maps.append(m)
    return in_maps


_NC_CACHE = {}


def kernel(**inputs) -> np.ndarray:
    in_maps = _prep_inputs(**inputs)
    if 'nc' not in _NC_CACHE:
        _NC_CACHE['nc'] = build_kernel(reps=1)
    nc = _NC_CACHE['nc']
    res = run_bass_kernel_spmd(nc, in_maps, core_ids=list(range(NCORES)))
    out = np.empty((B, 4), np.float32)
    for c in range(NCORES):
        out[c * ROWS:(c + 1) * ROWS, :] = res.results[c]["out"].T
    return out
